# revision 32
# baseline (speedup 1.0000x reference)
"""Causal self-attention (B=2, T=2048, C=1024, 16 heads) on 8 trn2 cores.

Sharding: core = (batch b, head-group hg) on a 2x4 grid; each core computes
QKV projection, causal attention and the partial c_proj for its 4 heads of
one batch element. Host sums the 4 partials per batch element (replaces the
all-reduce) and adds bproj.

Schedule (v2): t-block streaming with interleaved emission. The causal
structure means attention q-block qb only needs K/V tiles from t-blocks
<= qb, so projection of t-block tb+1 and c_proj of block tb are emitted as
"filler" matmuls INSIDE the exp-bound attention stretch of q-block tb,
keeping the PE busy while the scalar engine works through the softmax exps.

Per-head attention (S^T layout, [k,q] tiles):
  - Q^T/K^T stored as head-pair tiles [128, T] (heads 2i, 2i+1 stacked);
    score matmuls contract over the head's 64 channels directly (no
    zero-padding).
  - Score pairs land in [128, 1024] PSUM tiles (2 banks) so below-diagonal
    exps fuse into one [128, 1024] ACTIVATE (fewer scalar-engine
    instructions); diagonal tiles get an additive DVE triangle mask first.
  - V tiles carry a fused ones column per head so the AV matmul's row 64
    accumulates the softmax denominator; V-bias is pre-added to V (exact:
    (P @ (V + 1 b^T)) / s == P@V/s + b since rows of P/s sum to 1).
  - Normalization: O^T+denominator copied PSUM->SBUF right after the AV
    chain (releases the PSUM bank fast), then a finisher deferred by one
    head computes 1/s = Exp(-Ln(s)) on the scalar engine (Ln and Exp share
    the natural_log_exp activation table -- no 1.3us table reloads),
    broadcasts it across partitions with a single DRAM-bounce DMA pair and
    applies one DVE multiply into y^T.
"""

import os
import sys
import types
from collections import deque
from contextlib import ExitStack

import numpy as np

# ---------------------------------------------------------------------------
# Environment compatibility (self-contained on purpose).
# ---------------------------------------------------------------------------


def _install_axon_ntff_hook():
    """Provide the missing ``antenv.axon_hooks`` module so that
    ``run_bass_kernel_spmd(trace=True)`` works under axon in this container."""
    if "antenv.axon_hooks" in sys.modules:
        return
    try:
        import antenv
    except ImportError:
        return
    mod = types.ModuleType("antenv.axon_hooks")
    holder = [None]
    mod.set_axon_ntff_profile_hook = lambda h: holder.__setitem__(0, h)
    mod.get_axon_ntff_profile_hook = lambda: holder[0]
    sys.modules["antenv.axon_hooks"] = mod
    antenv.axon_hooks = mod
    try:
        from trn_agent_boot.trn_boot import _ntff_profile_via_ctypes

        hook = _ntff_profile_via_ctypes("/opt/axon/libaxon_pjrt.so")
        if hook is not None:
            mod.set_axon_ntff_profile_hook(hook)
    except Exception:
        pass


_install_axon_ntff_hook()

import concourse.bass as bass  # noqa: E402
import concourse.mybir as mybir  # noqa: E402
import concourse.tile as tile  # noqa: E402
from concourse.bass_utils import run_bass_kernel_spmd  # noqa: E402


def _split_multi_waits(nc, max_waits=1):
    """The walrus build here rejects instructions with more than one sync
    wait; move excess waits onto same-engine NoOps placed just before the
    instruction (sequential waiting is equivalent for monotonic sems)."""
    n = 0
    for func in nc.m.functions:
        for bb in func.blocks:
            out = []
            changed = False
            for inst in bb.instructions:
                si = inst.sync_info
                waits = list(si.on_wait) if si is not None and si.on_wait else []
                if len(waits) > max_waits:
                    changed = True
                    extra, keep = waits[:-max_waits], waits[-max_waits:]
                    for i in range(0, len(extra), max_waits):
                        n += 1
                        out.append(
                            mybir.InstNoOp(
                                name=f"{inst.name}-ws{i}",
                                engine=inst.engine,
                                ins=[],
                                outs=[],
                                sync_info=mybir.SyncInfo(
                                    on_wait=extra[i : i + max_waits], on_update=[]
                                ),
                                text_hint="wait_split",
                            )
                        )
                    si.on_wait = keep
                out.append(inst)
            if changed:
                bb.instructions = out
    return n


# ---------------------------------------------------------------------------
# Problem constants (hardcoded per spec).
# ---------------------------------------------------------------------------

B, T, C = 2, 2048, 1024
N_HEAD = 16
D = 64  # head dim
N_CORES = 8
HG = 4  # head groups (cores per batch element)
NH = N_HEAD // HG  # heads per core = 4
HD = NH * D  # head channels per core = 256
CK = C // 128  # contraction chunks = 8
TT = T // 128  # t tiles = 16
QB = T // 512  # q blocks = 4
NG = HD // 128  # head-pair groups per core = 2

F32 = mybir.dt.float32
_MM_CHOICES = {
    "fp32": mybir.dt.float32,
    "fp32r": mybir.dt.float32r,
    "bf16": mybir.dt.bfloat16,
}
MM_DT = _MM_CHOICES[os.environ.get("KERNEL_MM_DT", "bf16")]
MM_NP = mybir.dt.np(MM_DT)
OUT_BF16 = os.environ.get("KERNEL_OUT_BF16", "1") == "1"
OUT_DT = MM_DT if OUT_BF16 else F32
OUT_NP = mybir.dt.np(OUT_DT)
TICK_N = int(os.environ.get("KERNEL_TICK_N", "3"))

NEG = -1.0e9

TRACE = False
LAST_RESULT = None
_NC_CACHE = {}


def _build_nc():
    nc = bass.Bass("TRN2", target_bir_lowering=False)

    xT = nc.dram_tensor("xT", [C, T], MM_DT, kind="ExternalInput")
    ones_d = nc.inline_tensor(np.ones((1, 64), mybir.dt.np(MM_DT)), name="ones64")
    eye4_d = nc.inline_tensor(np.eye(4, dtype=np.float32), name="eye4")
    wq = nc.dram_tensor("wq", [C, HD], MM_DT, kind="ExternalInput")
    wk = nc.dram_tensor("wk", [C, HD], MM_DT, kind="ExternalInput")
    wv = nc.dram_tensor("wv", [C, HD], MM_DT, kind="ExternalInput")
    bqk = nc.dram_tensor("bqk", [2 * NG, 128], F32, kind="ExternalInput")
    bv = nc.dram_tensor("bv", [HD], F32, kind="ExternalInput")
    wp = nc.dram_tensor("wp", [HD, C], MM_DT, kind="ExternalInput")
    out = nc.dram_tensor("out", [T, C], OUT_DT, kind="ExternalOutput")

    with tile.TileContext(nc) as tc:
        _emit(nc, tc, xT, wq, wk, wv, bqk, bv, wp, out, ones_d, eye4_d)

    _split_multi_waits(nc)
    return nc


def _emit(nc, tc, xT, wq, wk, wv, bqk, bv, wp, out, ones_d, eye4_d):
    ctx = ExitStack()
    with ctx:
        consts = ctx.enter_context(tc.tile_pool(name="consts", bufs=1))
        xt_pool = ctx.enter_context(tc.tile_pool(name="xt", bufs=1))
        qt_pool = ctx.enter_context(tc.tile_pool(name="qt", bufs=NG))
        kt_pool = ctx.enter_context(tc.tile_pool(name="kt", bufs=NG))
        vo_pool = ctx.enter_context(tc.tile_pool(name="vo", bufs=TT))
        yt_pool = ctx.enter_context(tc.tile_pool(name="yt", bufs=NG))
        pt_pool = ctx.enter_context(tc.tile_pool(name="pt", bufs=8))
        yun_pool = ctx.enter_context(tc.tile_pool(name="yun", bufs=3))
        rb_pool = ctx.enter_context(tc.tile_pool(name="rb", bufs=2))
        ob_pool = ctx.enter_context(tc.tile_pool(name="ob", bufs=3))
        dram = ctx.enter_context(tc.tile_pool(name="dram", bufs=2, space="DRAM"))
        p_qk = ctx.enter_context(tc.tile_pool(name="p_qk", bufs=2, space="PSUM"))
        p_st = ctx.enter_context(tc.tile_pool(name="p_st", bufs=2, space="PSUM"))
        p_ot = ctx.enter_context(tc.tile_pool(name="p_ot", bufs=2, space="PSUM"))

        # ---- constant + input loads ----------------------------------------
        # Weights stream on the SP DGE queue, xT t-blocks on the gpsimd queue
        # (one big DMA per t-block; tb=0 split across both queues) so the
        # first projection can start as soon as the preamble ends.
        w_sb = {}
        xt_all = xt_pool.tile([128, CK, T], MM_DT, tag="xt", name="xt")
        xr = xT.rearrange("(o p) t -> p o t", p=128)

        # Q/K biases arrive as [4,128] (fat DMA descriptors -- a [128,4]
        # transfer is 128 8-byte descriptors and takes ~11us!) and are
        # transposed onto partitions with a tiny identity matmul. Their tiny
        # DMAs go first so the leading bias matmul never stalls the PE queue.
        bqk_r = consts.tile([2 * NG, 128], F32, tag="bqk_r")
        nc.sync.dma_start(bqk_r[:], bqk[:])
        eye4_sb = consts.tile([2 * NG, 2 * NG], F32, tag="eye4")
        nc.sync.dma_start(eye4_sb[:], eye4_d[:])
        wq_t = consts.tile([128, CK, HD], MM_DT, tag="wq", name="wq")
        nc.sync.dma_start(wq_t[:], wq.rearrange("(o p) n -> p o n", p=128))
        w_sb["wq"] = wq_t
        nc.gpsimd.dma_start(xt_all[:, 0 : CK // 2, 0:512], xr[:, 0 : CK // 2, 0:512])
        nc.sync.dma_start(
            xt_all[:, CK // 2 : CK, 0:512], xr[:, CK // 2 : CK, 0:512]
        )
        for name, w in (("wk", wk), ("wv", wv)):
            t = consts.tile([128, CK, HD], MM_DT, tag=name, name=name)
            nc.sync.dma_start(t[:], w.rearrange("(o p) n -> p o n", p=128))
            w_sb[name] = t
        bias_sb = consts.tile([128, 2 * NG], F32, tag="bias")
        bias_ps = p_qk.tile([128, 512], F32, tag="pq")
        nc.tensor.matmul(
            bias_ps[:, 0 : 2 * NG], bqk_r[:], eye4_sb[:], start=True, stop=True
        )
        nc.vector.tensor_copy(bias_sb[:], bias_ps[:, 0 : 2 * NG])
        wp_sb = consts.tile([128, NG, C], MM_DT, tag="wp")
        nc.sync.dma_start(wp_sb[:], wp.rearrange("(o p) n -> p o n", p=128))
        ones_sb = consts.tile([1, 64], MM_DT, tag="ones")
        nc.sync.dma_start(ones_sb[:], ones_d[:])

        def emit_x_dma(tb):
            tbc = slice(tb * 512, (tb + 1) * 512)
            nc.gpsimd.dma_start(xt_all[:, :, tbc], xr[:, :, tbc])

        emit_x_dma(1)

        # V bias tile: bv broadcast to all partitions (added to every V row;
        # exact because softmax rows sum to 1 after normalization).
        bvt = consts.tile([128, NH, D], F32, tag="bvt")
        nc.sync.dma_start(
            bvt[:].rearrange("p h c -> p (h c)"),
            bv.rearrange("(o n) -> o n", o=1).to_broadcast((128, HD)),
        )

        # additive causal triangle mask [k_rel, q_rel]: 0 where k<=q else NEG
        mask_sb = consts.tile([128, 128], F32, tag="mask")
        nc.gpsimd.memset(mask_sb[:], 0.0)
        nc.gpsimd.affine_select(
            out=mask_sb[:],
            in_=mask_sb[:],
            compare_op=mybir.AluOpType.is_ge,
            fill=NEG,
            base=0,
            pattern=[[1, 128]],
            channel_multiplier=-1,
        )

        # ---- persistent tiles ----------------------------------------------
        qt_sb = [
            qt_pool.tile([128, T], MM_DT, tag="qt", name=f"qt{i}") for i in range(NG)
        ]
        kt_sb = [
            kt_pool.tile([128, T], MM_DT, tag="kt", name=f"kt{i}") for i in range(NG)
        ]
        yt_sb = [
            yt_pool.tile([128, T], MM_DT, tag="yt", name=f"yt{g}") for g in range(NG)
        ]
        vo_sb = [None] * TT

        # ---- work-chain emitters -------------------------------------------
        def qk_chain(which, dst, bcol, tb, i):
            tbc = slice(tb * 512, (tb + 1) * 512)
            ps = p_qk.tile([128, 512], F32, tag="pq")
            for ck in range(CK):
                nc.tensor.matmul(
                    ps[:],
                    w_sb[which][:, ck, i * 128 : (i + 1) * 128],
                    xt_all[:, ck, tbc],
                    start=(ck == 0),
                    stop=(ck == CK - 1),
                )
            nc.vector.tensor_scalar(
                dst[i][:, tbc], ps[:], bias_sb[:, bcol + i : bcol + i + 1], None,
                mybir.AluOpType.add,
            )

        def v_chain(tt):
            t = vo_pool.tile([128, NH * 128], MM_DT, tag="vo", name=f"vo{tt}")
            vo_sb[tt] = t
            v4 = t[:].rearrange("p (h c) -> p h c", h=NH)
            nc.gpsimd.memset(v4[:, :, D + 1 :], 0.0)
            nc.gpsimd.memset(v4[:, :, D : D + 1], 1.0)
            ps = p_qk.tile([128, 512], F32, tag="pq")
            ttc = slice(tt * 128, (tt + 1) * 128)
            for ck in range(CK):
                nc.tensor.matmul(
                    ps[:, :HD],
                    xt_all[:, ck, ttc],
                    w_sb["wv"][:, ck, :],
                    start=(ck == 0),
                    stop=(ck == CK - 1),
                )
            nc.vector.tensor_tensor(
                v4[:, :, 0:D],
                ps[:, :HD].rearrange("p (h c) -> p h c", h=NH),
                bvt[:],
                mybir.AluOpType.add,
            )

        def cproj_chain(tt, nb):
            ps = p_qk.tile([128, 512], F32, tag="pq")
            nbc = slice(nb * 512, (nb + 1) * 512)
            for g in range(NG):
                nc.tensor.matmul(
                    ps[:],
                    yt_sb[g][:, tt * 128 : (tt + 1) * 128],
                    wp_sb[:, g, nbc],
                    start=(g == 0),
                    stop=(g == NG - 1),
                )
            ob = ob_pool.tile([128, 512], OUT_DT, tag="ob")
            nc.vector.tensor_copy(ob[:], ps[:])
            nc.sync.dma_start(out[tt * 128 : (tt + 1) * 128, nbc], ob[:])

        # ---- filler queues --------------------------------------------------
        # proj chains must drain before the next q-block's attention; cproj
        # chains can drain any time (mostly during the exp-heavy qb=3).
        proj_q = deque()
        cpr_q = deque()
        tick_ct = [0]

        def pop_filler():
            if proj_q:
                proj_q.popleft()()
            elif cpr_q:
                cpr_q.popleft()()

        def tick():
            tick_ct[0] += 1
            if tick_ct[0] % TICK_N == 0:
                pop_filler()

        def flush_proj():
            while proj_q:
                proj_q.popleft()()

        # ---- attention ------------------------------------------------------
        def attention_head(qb, h, on_first_pair=None):
            i, jb = h // 2, (h % 2) * 64
            q0 = qb * 512
            kd, qd = kt_sb[i], qt_sb[i]
            ot = p_ot.tile([128, 512], F32, tag="ot")
            n_kt = 4 * qb + 4
            pending = []

            def emit_av(kt, pt, off, c_av):
                nc.tensor.matmul(
                    ot[:, c_av:512],
                    vo_sb[kt][:, h * 128 : (h + 1) * 128],
                    pt[:, off + c_av : off + 512],
                    start=(kt == 0),
                    stop=(kt == n_kt - 1),
                )
                tick()

            for p in range(n_kt // 2):
                kta = 2 * p
                ja = kta - 4 * qb
                st = p_st.tile([128, 1024], F32, tag="st")
                pt = pt_pool.tile([128, 1024], MM_DT, tag="pt")
                if ja <= 0:
                    # below-diagonal pairs and the (j0,j1) pair: full
                    # 512-wide score matmuls, one fused exp over both banks.
                    # For j1, cols [0:128) of its half are fully masked but
                    # merely skipped by the AV read (c_av=128), so exp-ing
                    # their unmasked finite values is harmless.
                    for z in range(2):
                        kt = kta + z
                        nc.tensor.matmul(
                            st[:, z * 512 : z * 512 + 512],
                            kd[jb : jb + 64, kt * 128 : (kt + 1) * 128],
                            qd[jb : jb + 64, q0 : q0 + 512],
                            start=True,
                            stop=True,
                        )
                        j = kt - 4 * qb
                        if j >= 0:
                            # diagonal window: additive triangle mask on PSUM
                            cm = z * 512 + 128 * j
                            nc.vector.tensor_tensor(
                                st[:, cm : cm + 128],
                                st[:, cm : cm + 128],
                                mask_sb[:],
                                mybir.AluOpType.add,
                            )
                    nc.scalar.activation(
                        pt[:, 0:1024],
                        st[:, 0:1024],
                        mybir.ActivationFunctionType.Exp,
                        scale=0.125,
                    )
                    pending.append((kta, pt, 0, 0))
                    pending.append((kta + 1, pt, 512, 128 if ja == 0 else 0))
                else:
                    # the (j2,j3) diagonal pair: trimmed matmuls + per-half exp
                    for z in range(2):
                        kt = kta + z
                        j = kt - 4 * qb
                        off, c = z * 512, 128 * j
                        nc.tensor.matmul(
                            st[:, off + c : off + 512],
                            kd[jb : jb + 64, kt * 128 : (kt + 1) * 128],
                            qd[jb : jb + 64, q0 + c : q0 + 512],
                            start=True,
                            stop=True,
                        )
                        nc.vector.tensor_tensor(
                            st[:, off + c : off + c + 128],
                            st[:, off + c : off + c + 128],
                            mask_sb[:],
                            mybir.AluOpType.add,
                        )
                        nc.scalar.activation(
                            pt[:, off + c : off + 512],
                            st[:, off + c : off + 512],
                            mybir.ActivationFunctionType.Exp,
                            scale=0.125,
                        )
                        pending.append((kt, pt, off, c))
                if p == 0 and on_first_pair is not None:
                    on_first_pair()
                while len(pending) > 4:
                    emit_av(*pending.pop(0))
            for e in pending:
                emit_av(*e)

            # O^T + denominator out of PSUM immediately (frees the ot bank).
            yun = yun_pool.tile([128, 512], F32, tag="yun")
            nc.vector.tensor_copy(yun[:], ot[:])

            def finish_part1(fast=False):
                # normalize: row 64 of ot is the softmax denominator along
                # q; 1/s = Exp(-Ln(s)) stays within one activation table.
                # The reciprocal row is broadcast across 64 partitions via a
                # DRAM-bounce DMA pair (no PE cost) or, for the
                # tail-critical last head, via a PE outer product with a
                # ones column (short serial chain).
                lnr = rb_pool.tile([1, 512], F32, tag="ln")
                nc.scalar.activation(
                    lnr[:], ot[64:65, :], mybir.ActivationFunctionType.Ln
                )
                if fast:
                    rr = rb_pool.tile([1, 512], MM_DT, tag="rrb")
                    nc.scalar.activation(
                        rr[:], lnr[:], mybir.ActivationFunctionType.Exp,
                        scale=-1.0,
                    )
                    rbx = p_ot.tile([128, 512], F32, tag="ot")
                    nc.tensor.matmul(
                        rbx[0:64, :], ones_sb[:], rr[:], start=True, stop=True
                    )
                else:
                    rr = rb_pool.tile([1, 512], F32, tag="rr")
                    nc.scalar.activation(
                        rr[:], lnr[:], mybir.ActivationFunctionType.Exp,
                        scale=-1.0,
                    )
                    rd = dram.tile([1, 512], F32, tag="rd")
                    nc.gpsimd.dma_start(rd[:], rr[:])
                    rbx = rb_pool.tile([64, 512], F32, tag="rb")
                    nc.gpsimd.dma_start(rbx[:], rd[:].to_broadcast((64, 512)))

                def finish_part2():
                    # emitted late so this DVE op (waiting on the broadcast)
                    # does not block the next head's mask adds in the
                    # in-order DVE queue.
                    nc.vector.tensor_tensor(
                        yt_sb[i][jb : jb + 64, q0 : q0 + 512],
                        yun[0:64, :],
                        rbx[0:64, :] if fast else rbx[:],
                        mybir.AluOpType.mult,
                    )

                return finish_part2

            return finish_part1

        # ---- schedule --------------------------------------------------------
        # proj(0) emitted inline, split so heads 0/1 of qb=0 start after the
        # first head-pair's Q/K land. Finishers are deferred by one head so
        # their cross-engine chain latency hides under the next head's work;
        # the last finisher of a q-block runs before its cproj chains are
        # queued (emission-order correctness for the yt reads).
        pending_p1 = [None]  # previous head's finish_part1 (scalar + DMA)
        pending_p2 = [None]  # its finish_part2 (the DVE multiply)

        def run_head(qb, h):
            def fire_prev_p1():
                if pending_p1[0] is not None:
                    pending_p2[0] = pending_p1[0]()
                    pending_p1[0] = None

            f = attention_head(qb, h, on_first_pair=fire_prev_p1)
            fire_prev_p1()
            if pending_p2[0] is not None:
                pending_p2[0]()
                pending_p2[0] = None
            pending_p1[0] = f

        def drain_fin(fast=False):
            if pending_p1[0] is not None:
                pending_p1[0](fast=fast)()
                pending_p1[0] = None

        def push_proj(tb):
            for i in range(NG):
                proj_q.append(lambda i=i, tb=tb: qk_chain("wq", qt_sb, 0, tb, i))
                proj_q.append(lambda i=i, tb=tb: qk_chain("wk", kt_sb, NG, tb, i))
            for tt in range(4 * tb, 4 * tb + 4):
                proj_q.append(lambda tt=tt: v_chain(tt))

        qk_chain("wq", qt_sb, 0, 0, 0)
        qk_chain("wk", kt_sb, NG, 0, 0)
        for tt in range(4):
            v_chain(tt)
        emit_x_dma(2)
        emit_x_dma(3)

        for qb in range(QB):
            if qb == 0:
                run_head(0, 0)
                run_head(0, 1)
                qk_chain("wq", qt_sb, 0, 0, 1)
                qk_chain("wk", kt_sb, NG, 0, 1)
                push_proj(1)
                run_head(0, 2)
                run_head(0, 3)
            else:
                if qb + 1 < QB:
                    push_proj(qb + 1)
                for h in range(NH):
                    run_head(qb, h)
            drain_fin(fast=(qb == QB - 1))
            flush_proj()
            for tt in range(qb * 4, qb * 4 + 4):
                for nb in range(C // 512):
                    cpr_q.append(lambda tt=tt, nb=nb: cproj_chain(tt, nb))

        while cpr_q:
            cpr_q.popleft()()


def _get_nc():
    key = (str(MM_DT), str(OUT_DT), TICK_N)
    if key not in _NC_CACHE:
        _NC_CACHE[key] = _build_nc()
    return _NC_CACHE[key]


def kernel(x, Wqkv, bqkv, Wproj, bproj):
    global LAST_RESULT
    x = np.asarray(x, dtype=np.float32)
    Wqkv = np.asarray(Wqkv, dtype=np.float32)
    bqkv = np.asarray(bqkv, dtype=np.float32)
    Wproj = np.asarray(Wproj, dtype=np.float32)
    bproj = np.asarray(bproj, dtype=np.float32)

    nc = _get_nc()
    in_maps = []
    for core in range(N_CORES):
        b, hg = core // HG, core % HG
        cs, ce = hg * HD, (hg + 1) * HD
        in_maps.append(
            {
                "xT": np.ascontiguousarray(x[b].T.astype(MM_NP)),
                "wq": np.ascontiguousarray(Wqkv[:, cs:ce].astype(MM_NP)),
                "wk": np.ascontiguousarray(Wqkv[:, C + cs : C + ce].astype(MM_NP)),
                "wv": np.ascontiguousarray(
                    Wqkv[:, 2 * C + cs : 2 * C + ce].astype(MM_NP)
                ),
                "bqk": np.ascontiguousarray(
                    np.concatenate(
                        [
                            bqkv[cs:ce].reshape(NG, 128),
                            bqkv[C + cs : C + ce].reshape(NG, 128),
                        ]
                    ).astype(np.float32)
                ),
                "bv": np.ascontiguousarray(bqkv[2 * C + cs : 2 * C + ce]),
                "wp": np.ascontiguousarray(Wproj[cs:ce, :].astype(MM_NP)),
            }
        )

    res = run_bass_kernel_spmd(
        nc, in_maps, core_ids=list(range(N_CORES)), trace=TRACE
    )
    LAST_RESULT = res

    outp = np.empty((B, T, C), dtype=np.float32)
    for b in range(B):
        acc = res.results[b * HG]["out"].astype(np.float32)
        for hg in range(1, HG):
            acc = acc + res.results[b * HG + hg]["out"].astype(np.float32)
        outp[b] = acc + bproj
    return outp
